# revision 4
# baseline (speedup 1.0000x reference)
"""Causal self-attention (B=4, T=2048, C=1024, NH=16) on 8 trn2 NeuronCores.

Sharding: hybrid batch x head tensor-parallel. Core c handles batch b=c//2 and
heads [8*(c%2), 8*(c%2)+8) (feature slice of 512 of each of Q/K/V). Each core:
  - PE-transposes its x[b] slab to get x^T (contraction dim on partitions),
  - computes Q^T,K^T in [feature, token] layout and V in [token, feature] layout
    (with a ones-column appended per head for the softmax denominator),
  - runs causal attention per head in score-transposed layout S^T[k,q]
    (softmax without max-subtraction: scores ~ N(0,1), no overflow risk),
  - normalizes with the ones-column colsum and a matmul partition-broadcast,
  - applies its 512-row slice of w_proj to produce a partial output.
Host sums the two partial outputs per batch and adds b_proj.

Matmuls run as float32r (reduced-precision fp32, full PE rate at N>=512).
"""

import sys

for _p in ("/opt/trn_rl_repo",):
    if _p not in sys.path:
        sys.path.append(_p)

import numpy as np

import concourse.bacc as bacc
import concourse.bass as bass
import concourse.mybir as mybir
from concourse.masks import make_identity
from concourse.tile import TileContext

P = 128
B, T, C, NH, HD = 4, 2048, 1024, 16, 64
NCORES = 8
FH = 512            # features per core = 8 heads * HD
NHC = 8             # heads per core
QT = 512            # q-tile width
NQT = T // QT       # 4
NKT = T // P        # 16 k-tiles
CC = C // P         # 8 contraction chunks
FC = FH // P        # 4 feature chunks
DT = mybir.dt.float32
DTR = mybir.dt.float32r   # reduced-precision matmul dtype


def _r(ap):
    """Bitcast an AP to the matmul compute dtype."""
    return ap.bitcast(DTR)


def build_nc():
    nc = bacc.Bacc("TRN2", target_bir_lowering=False, debug=False,
                   num_devices=NCORES)
    xb = nc.dram_tensor("xb", [T, C], DT, kind="ExternalInput")
    wq = nc.dram_tensor("wq", [C, FH], DT, kind="ExternalInput")
    wk = nc.dram_tensor("wk", [C, FH], DT, kind="ExternalInput")
    wv = nc.dram_tensor("wv", [C, FH], DT, kind="ExternalInput")
    wp = nc.dram_tensor("wp", [FH, C], DT, kind="ExternalInput")
    bq = nc.dram_tensor("bq", [FH], DT, kind="ExternalInput")
    bk = nc.dram_tensor("bk", [FH], DT, kind="ExternalInput")
    bv = nc.dram_tensor("bv", [FH], DT, kind="ExternalInput")
    yp = nc.dram_tensor("yp", [T, C], DT, kind="ExternalOutput")

    Exp = mybir.ActivationFunctionType.Exp
    Ident = mybir.ActivationFunctionType.Identity

    with TileContext(nc) as tc:
        with (
            tc.tile_pool(name="const", bufs=1) as const,
            tc.tile_pool(name="big", bufs=1) as big,
        ):
            ident = const.tile([P, P], DT)
            make_identity(nc, ident[:])
            ones_t = const.tile([P, P], DT)
            nc.gpsimd.memset(ones_t[:], 0.0)
            nc.vector.tensor_scalar_add(_r(ones_t[:]), ones_t[:], 1.0)

            bq_t = const.tile([P, FC], DT)
            bk_t = const.tile([P, FC], DT)
            bq8 = const.tile([P, FC], DT)
            bv_row = const.tile([P, FH], DT)
            nc.sync.dma_start(bq_t[:], bq.ap().rearrange("(c p) -> p c", p=P))
            nc.sync.dma_start(bk_t[:], bk.ap().rearrange("(c p) -> p c", p=P))
            nc.sync.dma_start(bv_row[0:1, :], bv.ap().rearrange("(a f) -> a f", a=1))
            nc.vector.tensor_scalar_mul(bq8[:], bq_t[:], 0.125)

            qTt = big.tile([P, FC, T], DT)     # Q^T * (1/8), [feature, token]
            kTt = big.tile([P, FC, T], DT)     # K^T
            v65 = big.tile([P, NKT, NHC, HD + 1], DT)  # V (+bias) with ones col
            nc.gpsimd.memset(v65[:, :, :, HD:HD + 1], 0.0)
            nc.vector.tensor_scalar_add(_r(v65[:, :, :, HD:HD + 1]),
                                        v65[:, :, :, HD:HD + 1], 1.0)

            # broadcast v-bias along tokens: vb_bc[t, f] = bv[f]
            vb_bc = const.tile([P, FH], DT)
            with tc.tile_pool(name="vbps", bufs=1, space="PSUM") as vbps:
                vb_ps = vbps.tile([P, FH], DT)
                nc.tensor.matmul(vb_ps[:], lhsT=ones_t[0:1, :],
                                 rhs=bv_row[0:1, :], start=True, stop=True)
                nc.vector.tensor_copy(vb_bc[:], vb_ps[:])

            # ---------------- phase A: x^T + QKV projection ----------------
            with (
                tc.tile_pool(name="wqkv", bufs=1) as wpool,
                tc.tile_pool(name="xa", bufs=2) as xap,
                tc.tile_pool(name="xT", bufs=1) as xtp,
                tc.tile_pool(name="tps", bufs=2, space="PSUM") as tpsp,
                tc.tile_pool(name="qkvps", bufs=3, space="PSUM") as qps,
            ):
                wq_t = wpool.tile([P, CC, FH], DT, tag="wq")
                wk_t = wpool.tile([P, CC, FH], DT, tag="wk")
                wv_t = wpool.tile([P, CC, FH], DT, tag="wv")
                nc.sync.dma_start(_r(wq_t[:]), _r(wq.ap().rearrange("(cc p) f -> p cc f", p=P)))
                nc.sync.dma_start(_r(wk_t[:]), _r(wk.ap().rearrange("(cc p) f -> p cc f", p=P)))
                nc.sync.dma_start(_r(wv_t[:]), _r(wv.ap().rearrange("(cc p) f -> p cc f", p=P)))

                for tt in range(NQT):
                    xT = xtp.tile([P, CC, QT], DT, tag="xT")
                    for half in range(2):
                        xa = xap.tile([P, 2, C], DT, tag="xa")
                        rows = xb.ap()[tt * QT + half * 256:
                                       tt * QT + half * 256 + 256, :]
                        nc.sync.dma_start(xa[:], rows.rearrange("(ts p) c -> p ts c", p=P))
                        for ts in range(2):
                            for cc in range(CC):
                                pst = tpsp.tile([P, P], DT, tag="tps")
                                nc.tensor.transpose(
                                    pst[:], xa[:, ts, cc * P:(cc + 1) * P], ident[:])
                                off = (half * 2 + ts) * P
                                nc.vector.tensor_copy(
                                    _r(xT[:, cc, off:off + P]), pst[:])
                    # Q^T and K^T slices for these 512 tokens
                    for wt, dstT, bias_ap, scale in (
                        (wq_t, qTt, bq8, 0.125),
                        (wk_t, kTt, bk_t, 1.0),
                    ):
                        for fc in range(FC):
                            ps = qps.tile([P, QT], DT, tag="qkv")
                            for cc in range(CC):
                                nc.tensor.matmul(
                                    ps[:],
                                    lhsT=_r(wt[:, cc, fc * P:(fc + 1) * P]),
                                    rhs=_r(xT[:, cc, :]),
                                    start=(cc == 0), stop=(cc == CC - 1))
                            nc.scalar.activation(
                                _r(dstT[:, fc, tt * QT:(tt + 1) * QT]), ps[:],
                                Ident, bias=bias_ap[:, fc:fc + 1], scale=scale)
                    # V (natural layout) for these 512 tokens, + bias
                    for ts in range(4):
                        ps = qps.tile([P, FH], DT, tag="qkv")
                        for cc in range(CC):
                            nc.tensor.matmul(
                                ps[:],
                                lhsT=_r(xT[:, cc, ts * P:(ts + 1) * P]),
                                rhs=_r(wv_t[:, cc, :]),
                                start=(cc == 0), stop=(cc == CC - 1))
                        kt = tt * 4 + ts
                        nc.vector.tensor_tensor(
                            out=_r(v65[:, kt, :, 0:HD]),
                            in0=ps[:].rearrange("p (h d) -> p h d", d=HD),
                            in1=vb_bc[:].rearrange("p (h d) -> p h d", d=HD),
                            op=mybir.AluOpType.add)

            # ---------------- phase B: attention per head ----------------
            with (
                tc.tile_pool(name="bigB", bufs=1) as bigB,
                tc.tile_pool(name="wp", bufs=1) as wppool,
                tc.tile_pool(name="pb", bufs=4) as pbp,
                tc.tile_pool(name="nrm", bufs=2) as nrm,
                tc.tile_pool(name="sps", bufs=2, space="PSUM") as sps,
                tc.tile_pool(name="ops", bufs=2, space="PSUM") as ops,
                tc.tile_pool(name="bcps", bufs=2, space="PSUM") as bcps,
            ):
                attnT = bigB.tile([P, FC, T], DT)
                wp_t = wppool.tile([P, FC, C], DT)
                nc.sync.dma_start(_r(wp_t[:]), _r(wp.ap().rearrange("(fc p) o -> p fc o", p=P)))

                for h in range(NHC):
                    pb0 = HD * (h % 2)
                    fch = h // 2
                    for j in range(NQT):
                        qs = slice(j * QT, (j + 1) * QT)
                        O = ops.tile([P, QT], DT, tag="O")
                        nk = 4 * j + 4
                        for kt in range(nk):
                            S = sps.tile([P, QT], DT, tag="S")
                            nc.tensor.matmul(
                                S[:],
                                lhsT=_r(kTt[pb0:pb0 + HD, fch, kt * P:(kt + 1) * P]),
                                rhs=_r(qTt[pb0:pb0 + HD, fch, qs]),
                                start=True, stop=True)
                            Pt = pbp.tile([P, QT], DT, tag="P")
                            nc.scalar.activation(_r(Pt[:]), S[:], Exp)
                            if kt >= 4 * j:
                                # zero the strictly-above-diagonal region
                                delta = kt * P - j * QT
                                nc.gpsimd.affine_select(
                                    out=_r(Pt[:]), in_=_r(Pt[:]),
                                    pattern=[[1, QT]],
                                    compare_op=mybir.AluOpType.is_ge,
                                    fill=0.0, base=-delta,
                                    channel_multiplier=-1)
                            nc.tensor.matmul(
                                O[0:HD + 1, :],
                                lhsT=_r(v65[:, kt, h, :]),
                                rhs=_r(Pt[:]),
                                start=(kt == 0), stop=(kt == nk - 1))
                        # normalize: attnT[d, q] = O[d, q] / O[HD, q]
                        rsc = nrm.tile([P, QT], DT, tag="rsc")
                        with nc.allow_low_precision(reason="f32r rounding of softmax denom reciprocal"):
                            nc.vector.reciprocal(_r(rsc[HD:HD + 1, :]), O[HD:HD + 1, :])
                        bc = bcps.tile([P, QT], DT, tag="bc")
                        nc.tensor.matmul(
                            bc[0:HD, :], lhsT=_r(ones_t[HD:HD + 1, 0:HD]),
                            rhs=_r(rsc[HD:HD + 1, :]), start=True, stop=True)
                        bcs = nrm.tile([P, QT], DT, tag="bcs")
                        nc.vector.tensor_copy(bcs[0:HD, :], bc[0:HD, :])
                        if h % 2 == 0:
                            nc.vector.tensor_tensor(
                                out=_r(attnT[0:HD, fch, qs]), in0=O[0:HD, :],
                                in1=bcs[0:HD, :], op=mybir.AluOpType.mult)
                        else:
                            # engines cannot shift partition base; bounce via DMA
                            tmp = nrm.tile([P, QT], DT, tag="tmp")
                            nc.vector.tensor_tensor(
                                out=tmp[0:HD, :], in0=O[0:HD, :],
                                in1=bcs[0:HD, :], op=mybir.AluOpType.mult)
                            nc.sync.dma_start(_r(attnT[HD:2 * HD, fch, qs]), _r(tmp[0:HD, :]))

                # ---------------- phase C: output projection ----------------
                with (
                    tc.tile_pool(name="pps", bufs=2, space="PSUM") as pps,
                    tc.tile_pool(name="oc", bufs=2) as ocp,
                ):
                    for tt in range(T // P):
                        oc = ocp.tile([P, C], DT, tag="oc")
                        for co in range(2):
                            pp = pps.tile([P, QT], DT, tag="pp")
                            for fc in range(FC):
                                nc.tensor.matmul(
                                    pp[:],
                                    lhsT=_r(attnT[:, fc, tt * P:(tt + 1) * P]),
                                    rhs=_r(wp_t[:, fc, co * QT:(co + 1) * QT]),
                                    start=(fc == 0), stop=(fc == FC - 1))
                            nc.vector.tensor_copy(oc[:, co * QT:(co + 1) * QT], pp[:])
                        nc.sync.dma_start(yp.ap()[tt * P:(tt + 1) * P, :], oc[:])

    nc.finalize()
    return nc


# ---------------------------------------------------------------------------
# host side: cached jitted SPMD runner (mirrors bass2jax.run_bass_via_pjrt,
# but builds the jit once so repeat kernel() calls skip retracing/compile)
# ---------------------------------------------------------------------------

_RUNNER = None


def _make_runner():
    import jax
    import jax.numpy as jnp
    from jax.experimental.shard_map import shard_map
    from jax.sharding import Mesh, PartitionSpec

    from concourse import bass2jax

    nc = build_nc()
    bass2jax.install_neuronx_cc_hook()

    partition_name = (nc.partition_id_tensor.name
                      if nc.partition_id_tensor else None)
    in_names = []
    out_names = []
    out_avals = []
    out_shapes = []
    for alloc in nc.m.functions[0].allocations:
        if not isinstance(alloc, mybir.MemoryLocationSet):
            continue
        name = alloc.memorylocations[0].name
        if alloc.kind == "ExternalInput":
            if name != partition_name:
                in_names.append(name)
        elif alloc.kind == "ExternalOutput":
            shape = tuple(alloc.tensor_shape)
            dtype = mybir.dt.np(alloc.dtype)
            out_avals.append(jax.core.ShapedArray(shape, dtype))
            out_shapes.append((name, shape, dtype))
            out_names.append(name)
    n_params = len(in_names)
    n_outs = len(out_avals)
    all_in_names = list(in_names) + list(out_names)
    if partition_name is not None:
        all_in_names.append(partition_name)
    donate = tuple(range(n_params, n_params + n_outs))

    def _body(*args):
        operands = list(args)
        if partition_name is not None:
            operands.append(bass2jax.partition_id_tensor())
        outs = bass2jax._bass_exec_p.bind(
            *operands,
            out_avals=tuple(out_avals),
            in_names=tuple(all_in_names),
            out_names=tuple(out_names),
            lowering_input_output_aliases=(),
            sim_require_finite=True,
            sim_require_nnan=True,
            nc=nc,
        )
        return tuple(outs)

    devices = jax.devices()[:NCORES]
    mesh = Mesh(np.asarray(devices), ("core",))
    in_specs = (PartitionSpec("core"),) * (n_params + n_outs)
    out_specs = (PartitionSpec("core"),) * n_outs
    sharded = jax.jit(
        shard_map(_body, mesh=mesh, in_specs=in_specs, out_specs=out_specs,
                  check_rep=False),
        donate_argnums=donate, keep_unused=True)

    def run(in_maps):
        concat_in = [
            np.concatenate([np.asarray(in_maps[c][name]) for c in range(NCORES)],
                           axis=0)
            for name in in_names
        ]
        concat_zeros = [
            np.zeros((NCORES * s[0], *s[1:]), dt) for (_, s, dt) in out_shapes
        ]
        out_arrs = sharded(*concat_in, *concat_zeros)
        return [
            {name: np.asarray(out_arrs[i]).reshape(NCORES, *shape)[c]
             for i, (name, shape, _) in enumerate(out_shapes)}
            for c in range(NCORES)
        ]

    return run, sharded, in_names, out_shapes, mesh


def _get_runner():
    global _RUNNER
    if _RUNNER is None:
        _RUNNER = _make_runner()
    return _RUNNER


def _in_maps(x, w_attn, b_attn, w_proj):
    maps = []
    for c in range(NCORES):
        b, j = c // 2, c % 2
        f0 = FH * j
        maps.append({
            "xb": np.ascontiguousarray(x[b]),
            "wq": np.ascontiguousarray(w_attn[:, f0:f0 + FH]),
            "wk": np.ascontiguousarray(w_attn[:, C + f0:C + f0 + FH]),
            "wv": np.ascontiguousarray(w_attn[:, 2 * C + f0:2 * C + f0 + FH]),
            "wp": np.ascontiguousarray(w_proj[f0:f0 + FH, :]),
            "bq": np.ascontiguousarray(b_attn[f0:f0 + FH]),
            "bk": np.ascontiguousarray(b_attn[C + f0:C + f0 + FH]),
            "bv": np.ascontiguousarray(b_attn[2 * C + f0:2 * C + f0 + FH]),
        })
    return maps


def kernel(x, w_attn, b_attn, w_proj, b_proj):
    x = np.asarray(x, dtype=np.float32)
    w_attn = np.asarray(w_attn, dtype=np.float32)
    b_attn = np.asarray(b_attn, dtype=np.float32)
    w_proj = np.asarray(w_proj, dtype=np.float32)
    b_proj = np.asarray(b_proj, dtype=np.float32)

    run, *_ = _get_runner()
    results = run(_in_maps(x, w_attn, b_attn, w_proj))

    y = np.empty((B, T, C), dtype=np.float32)
    for b in range(B):
        y[b] = results[2 * b]["yp"] + results[2 * b + 1]["yp"] + b_proj
    return y


# revision 14
# speedup vs baseline: 1.4497x; 1.4497x over previous
"""Causal self-attention (B=4, T=2048, C=1024, NH=16) on 8 trn2 NeuronCores.

Sharding: hybrid batch x head tensor-parallel. Core c handles batch b=c//2 and
heads [8*(c%2), 8*(c%2)+8) (feature slice of 512 of each of Q/K/V). Each core:
  - PE-transposes its x[b] slab to get x^T (contraction dim on partitions),
  - computes Q^T,K^T in [feature, token] layout and V in [token, feature] layout
    (with a ones-column appended per head for the softmax denominator),
  - runs causal attention per head in score-transposed layout S^T[k,q]
    (softmax without max-subtraction: scores ~ N(0,1), no overflow risk),
  - normalizes with the ones-column colsum and a matmul partition-broadcast,
  - applies its 512-row slice of w_proj to produce a partial output.
Host sums the two partial outputs per batch and adds b_proj.

Matmuls run as float32r (reduced-precision fp32, full PE rate at N>=512).
"""

import sys

for _p in ("/opt/trn_rl_repo",):
    if _p not in sys.path:
        sys.path.append(_p)

import numpy as np

import concourse.bacc as bacc
import concourse.bass as bass
import concourse.mybir as mybir
from concourse.masks import make_identity
from concourse.tile import TileContext

P = 128
B, T, C, NH, HD = 4, 2048, 1024, 16, 64
NCORES = 8
FH = 512            # features per core = 8 heads * HD
NHC = 8             # heads per core
QT = 512            # q-tile width
NQT = T // QT       # 4
NKT = T // P        # 16 k-tiles
CC = C // P         # 8 contraction chunks
FC = FH // P        # 4 feature chunks
DT = mybir.dt.float32
DTR = mybir.dt.float32r   # reduced-precision matmul dtype
DTB = mybir.dt.bfloat16   # attention P/V dtype


def _r(ap):
    """Bitcast an AP to the matmul compute dtype."""
    return ap.bitcast(DTR)


def build_nc():
    nc = bacc.Bacc("TRN2", target_bir_lowering=False, debug=False,
                   num_devices=NCORES)
    xb = nc.dram_tensor("xb", [T, C], DT, kind="ExternalInput")
    wq = nc.dram_tensor("wq", [C, FH], DT, kind="ExternalInput")
    wk = nc.dram_tensor("wk", [C, FH], DT, kind="ExternalInput")
    wv = nc.dram_tensor("wv", [C, FH], DT, kind="ExternalInput")
    wp = nc.dram_tensor("wp", [FH, C], DT, kind="ExternalInput")
    bq = nc.dram_tensor("bq", [FH], DT, kind="ExternalInput")
    bk = nc.dram_tensor("bk", [FH], DT, kind="ExternalInput")
    bv = nc.dram_tensor("bv", [FH], DT, kind="ExternalInput")
    yp = nc.dram_tensor("yp", [T, C], DT, kind="ExternalOutput")

    Exp = mybir.ActivationFunctionType.Exp
    Ident = mybir.ActivationFunctionType.Identity

    with TileContext(nc) as tc:
        with (
            tc.tile_pool(name="const", bufs=1) as const,
            tc.tile_pool(name="big", bufs=1) as big,
            tc.tile_pool(name="wqkv", bufs=1) as wpool,
            tc.tile_pool(name="qTp", bufs=2) as qTp,
            tc.tile_pool(name="atp", bufs=2) as atp,
            tc.tile_pool(name="xa", bufs=2) as xap,
            tc.tile_pool(name="xT", bufs=1) as xtp,
            tc.tile_pool(name="pb", bufs=4) as pbp,
            tc.tile_pool(name="nrm", bufs=2) as nrm,
            tc.tile_pool(name="oc", bufs=2) as ocp,
            tc.tile_pool(name="qkvps", bufs=2, space="PSUM") as qps,
            tc.tile_pool(name="sps", bufs=2, space="PSUM") as sps,
            tc.tile_pool(name="ops", bufs=1, space="PSUM") as ops,
        ):
            ident = const.tile([P, P], DT)
            make_identity(nc, ident[:])
            ones_t = const.tile([P, P], DT)
            nc.gpsimd.memset(ones_t[:], 0.0)
            nc.vector.tensor_scalar_add(_r(ones_t[:]), ones_t[:], 1.0)

            bq_t = const.tile([P, FC], DT)
            bk_t = const.tile([P, FC], DT)
            bq8 = const.tile([P, FC], DT)
            bv_row = const.tile([P, FH], DT)
            nc.sync.dma_start(bq_t[:], bq.ap().rearrange("(c p) -> p c", p=P))
            nc.sync.dma_start(bk_t[:], bk.ap().rearrange("(c p) -> p c", p=P))
            nc.sync.dma_start(bv_row[0:1, :], bv.ap().rearrange("(a f) -> a f", a=1))
            nc.vector.tensor_scalar_mul(bq8[:], bq_t[:], 0.125)

            kTt = big.tile([P, FC, T], DT)             # K^T (f32r-rounded)
            v65 = big.tile([P, NKT, NHC, HD + 1], DTB)  # V (+bias), ones col, bf16
            nc.gpsimd.memset(v65[:, :, :, HD:HD + 1], 1.0)

            wq_t = wpool.tile([P, CC, FH], DT, tag="wq")
            wk_t = wpool.tile([P, CC, FH], DT, tag="wk")
            wv_t = wpool.tile([P, CC, FH], DT, tag="wv")
            wp_t = wpool.tile([P, FC, C], DT, tag="wp")
            for cc in range(CC):
                nc.sync.dma_start(
                    _r(wq_t[:, cc, :]),
                    _r(wq.ap()[cc * P:(cc + 1) * P, :]))
                nc.sync.dma_start(
                    _r(wk_t[:, cc, :]),
                    _r(wk.ap()[cc * P:(cc + 1) * P, :]))
            for cc in range(CC):
                nc.sync.dma_start(
                    _r(wv_t[:, cc, :]),
                    _r(wv.ap()[cc * P:(cc + 1) * P, :]))
            for fc in range(FC):
                nc.sync.dma_start(
                    _r(wp_t[:, fc, :]),
                    _r(wp.ap()[fc * P:(fc + 1) * P, :]))

            # broadcast v-bias along tokens: vb_bc[t, f] = bv[f]
            vb_bc = const.tile([P, FH], DT)
            vb_ps = qps.tile([P, FH], DT, tag="qkv")
            nc.tensor.matmul(vb_ps[:], lhsT=ones_t[0:1, :],
                             rhs=bv_row[0:1, :], start=True, stop=True)
            nc.vector.tensor_copy(vb_bc[:], vb_ps[:])

            def qkv_chunk(tt, qT):
                """x^T + Q^T/K^T/V for tokens [tt*512, (tt+1)*512)."""
                xT = xtp.tile([P, CC, QT], DT, tag="xT")
                for to in range(4):        # 128-token blocks
                    xa = xap.tile([P, C], DT, tag="xa")
                    rows = xb.ap()[tt * QT + to * P: tt * QT + (to + 1) * P, :]
                    nc.scalar.dma_start(xa[:], rows)
                    for g in range(2):     # c-chunk groups of 4
                        pst = qps.tile([P, QT], DT, tag="qkv", name=f"pst_{to}_{g}")
                        for cc4 in range(4):
                            cc = g * 4 + cc4
                            nc.tensor.transpose(
                                pst[:, cc4 * P:(cc4 + 1) * P],
                                xa[:, cc * P:(cc + 1) * P], ident[:])
                        nc.vector.tensor_copy(
                            _r(xT[:, g * 4:g * 4 + 4, to * P:(to + 1) * P]),
                            pst[:].rearrange("p (c t) -> p c t", t=P))
                for wt, dst, bias_ap, scale in (
                    (wq_t, ("q", qT), bq8, 0.125),
                    (wk_t, ("k", None), bk_t, 1.0),
                ):
                    for fc in range(FC):
                        ps = qps.tile([P, QT], DT, tag="qkv")
                        for cc in range(CC):
                            nc.tensor.matmul(
                                ps[:],
                                lhsT=_r(wt[:, cc, fc * P:(fc + 1) * P]),
                                rhs=_r(xT[:, cc, :]),
                                start=(cc == 0), stop=(cc == CC - 1))
                        if dst[0] == "q":
                            out_ap = dst[1][:, fc, :]
                        else:
                            out_ap = kTt[:, fc, tt * QT:(tt + 1) * QT]
                        nc.scalar.activation(
                            _r(out_ap), ps[:],
                            Ident, bias=bias_ap[:, fc:fc + 1], scale=scale)
                for ts in range(4):
                    ps = qps.tile([P, FH], DT, tag="qkv")
                    for cc in range(CC):
                        nc.tensor.matmul(
                            ps[:],
                            lhsT=_r(xT[:, cc, ts * P:(ts + 1) * P]),
                            rhs=_r(wv_t[:, cc, :]),
                            start=(cc == 0), stop=(cc == CC - 1))
                    kt = tt * 4 + ts
                    nc.vector.tensor_tensor(
                        out=v65[:, kt, :, 0:HD],
                        in0=ps[:].rearrange("p (h d) -> p h d", d=HD),
                        in1=vb_bc[:].rearrange("p (h d) -> p h d", d=HD),
                        op=mybir.AluOpType.add)

            def attn_pair(i, j, qT, atn):
                """Heads (2i, 2i+1) for q-tile j; diagonal blocks first.

                Both heads' scores live in one [P, 1024] psum (2 banks, one
                per head) so exp runs as a single double-width ACT op.
                """
                heads = ((2 * i, 0), (2 * i + 1, HD))
                O = ops.tile([P, 2 * QT], DT, tag="O", name=f"O_{i}_{j}")
                nk = 4 * j + 4
                kts = [4 * j, 4 * j + 1, 4 * j + 2, 4 * j + 3] + list(range(4 * j))
                for idx, kt in enumerate(kts):
                    delta = max(0, kt * P - j * QT)
                    S = sps.tile([P, 2 * QT], DT, tag="S", name=f"S_{i}_{kt}")
                    for hh, (h, pb0) in enumerate(heads):
                        nc.tensor.matmul(
                            S[:, hh * QT + delta:(hh + 1) * QT],
                            lhsT=_r(kTt[pb0:pb0 + HD, i, kt * P:(kt + 1) * P]),
                            rhs=_r(qT[pb0:pb0 + HD, i, delta:QT]),
                            start=True, stop=True)
                    Pt = pbp.tile([P, 2 * QT], DTB, tag="P")
                    nc.scalar.activation(
                        Pt[:].rearrange("p (h w) -> p h w", h=2)[:, :, delta:QT],
                        S[:].rearrange("p (h w) -> p h w", h=2)[:, :, delta:QT],
                        Exp)
                    for hh, (h, pb0) in enumerate(heads):
                        if kt >= 4 * j:
                            nc.gpsimd.affine_select(
                                out=Pt[:, hh * QT + delta:hh * QT + delta + P],
                                in_=Pt[:, hh * QT + delta:hh * QT + delta + P],
                                pattern=[[1, P]],
                                compare_op=mybir.AluOpType.is_ge,
                                fill=0.0, base=0,
                                channel_multiplier=-1)
                        nc.tensor.matmul(
                            O[0:HD + 1, hh * QT + delta:(hh + 1) * QT],
                            lhsT=v65[:, kt, h, :],
                            rhs=Pt[:, hh * QT + delta:(hh + 1) * QT],
                            start=(idx == 0), stop=(idx == nk - 1))
                # copy O out of psum promptly so the single O buffer frees
                ocp_t = nrm.tile([P, 2 * QT], DT, tag="Ocpy")
                nc.vector.tensor_copy(_r(ocp_t[0:HD + 1, :]), O[0:HD + 1, :])
                for hh, (h, pb0) in enumerate(heads):
                    hs = slice(hh * QT, (hh + 1) * QT)
                    with nc.allow_low_precision(reason="f32r rounding of softmax denom reciprocal"):
                        nc.vector.reciprocal(_r(ocp_t[HD:HD + 1, hs]),
                                             ocp_t[HD:HD + 1, hs])
                    bc = qps.tile([P, QT], DT, tag="qkv", name=f"bc_{i}_{j}_{hh}")
                    nc.tensor.matmul(
                        bc[0:HD, :], lhsT=_r(ones_t[HD:HD + 1, 0:HD]),
                        rhs=_r(ocp_t[HD:HD + 1, hs]), start=True, stop=True)
                    if pb0 == 0:
                        nc.vector.tensor_tensor(
                            out=_r(atn[0:HD, i, :]), in0=bc[0:HD, :],
                            in1=ocp_t[0:HD, hs], op=mybir.AluOpType.mult)
                    else:
                        # engines cannot shift partition base; bounce via DMA
                        nc.vector.tensor_tensor(
                            out=_r(ocp_t[0:HD, hs]), in0=bc[0:HD, :],
                            in1=ocp_t[0:HD, hs], op=mybir.AluOpType.mult)
                        nc.gpsimd.dma_start(_r(atn[HD:2 * HD, i, :]),
                                            _r(ocp_t[0:HD, hs]))

            def proj_chunk(tt, atn):
                """Output projection for tokens [tt*512, (tt+1)*512)."""
                for t4 in range(4):
                    t_ = tt * 4 + t4
                    for co in range(2):
                        pp = qps.tile([P, QT], DT, tag="qkv")
                        for fc in range(FC):
                            nc.tensor.matmul(
                                pp[:],
                                lhsT=_r(atn[:, fc, t4 * P:(t4 + 1) * P]),
                                rhs=_r(wp_t[:, fc, co * QT:(co + 1) * QT]),
                                start=(fc == 0), stop=(fc == FC - 1))
                        oc = ocp.tile([P, QT], DT, tag="oc")
                        nc.vector.tensor_copy(oc[:], pp[:])
                        nc.scalar.dma_start(
                            yp.ap()[t_ * P:(t_ + 1) * P, co * QT:(co + 1) * QT],
                            oc[:])

            for tt in range(NQT):
                qT = qTp.tile([P, FC, QT], DT, tag="qT", name=f"qT_{tt}")
                atn = atp.tile([P, FC, QT], DT, tag="atn", name=f"atn_{tt}")
                qkv_chunk(tt, qT)
                for i in range(4):
                    attn_pair(i, j=tt, qT=qT, atn=atn)
                proj_chunk(tt, atn)

    nc.finalize()
    return nc


# ---------------------------------------------------------------------------
# host side: cached jitted SPMD runner (mirrors bass2jax.run_bass_via_pjrt,
# but builds the jit once so repeat kernel() calls skip retracing/compile)
# ---------------------------------------------------------------------------

_RUNNER = None


def _make_runner():
    import jax
    import jax.numpy as jnp
    from jax.experimental.shard_map import shard_map
    from jax.sharding import Mesh, PartitionSpec

    from concourse import bass2jax

    nc = build_nc()
    bass2jax.install_neuronx_cc_hook()

    partition_name = (nc.partition_id_tensor.name
                      if nc.partition_id_tensor else None)
    in_names = []
    out_names = []
    out_avals = []
    out_shapes = []
    for alloc in nc.m.functions[0].allocations:
        if not isinstance(alloc, mybir.MemoryLocationSet):
            continue
        name = alloc.memorylocations[0].name
        if alloc.kind == "ExternalInput":
            if name != partition_name:
                in_names.append(name)
        elif alloc.kind == "ExternalOutput":
            shape = tuple(alloc.tensor_shape)
            dtype = mybir.dt.np(alloc.dtype)
            out_avals.append(jax.core.ShapedArray(shape, dtype))
            out_shapes.append((name, shape, dtype))
            out_names.append(name)
    n_params = len(in_names)
    n_outs = len(out_avals)
    all_in_names = list(in_names) + list(out_names)
    if partition_name is not None:
        all_in_names.append(partition_name)
    donate = tuple(range(n_params, n_params + n_outs))

    def _body(*args):
        operands = list(args)
        if partition_name is not None:
            operands.append(bass2jax.partition_id_tensor())
        outs = bass2jax._bass_exec_p.bind(
            *operands,
            out_avals=tuple(out_avals),
            in_names=tuple(all_in_names),
            out_names=tuple(out_names),
            lowering_input_output_aliases=(),
            sim_require_finite=True,
            sim_require_nnan=True,
            nc=nc,
        )
        return tuple(outs)

    devices = jax.devices()[:NCORES]
    mesh = Mesh(np.asarray(devices), ("core",))
    in_specs = (PartitionSpec("core"),) * (n_params + n_outs)
    out_specs = (PartitionSpec("core"),) * n_outs
    sharded = jax.jit(
        shard_map(_body, mesh=mesh, in_specs=in_specs, out_specs=out_specs,
                  check_rep=False),
        donate_argnums=donate, keep_unused=True)

    def run(in_maps):
        concat_in = [
            np.concatenate([np.asarray(in_maps[c][name]) for c in range(NCORES)],
                           axis=0)
            for name in in_names
        ]
        concat_zeros = [
            np.zeros((NCORES * s[0], *s[1:]), dt) for (_, s, dt) in out_shapes
        ]
        out_arrs = sharded(*concat_in, *concat_zeros)
        return [
            {name: np.asarray(out_arrs[i]).reshape(NCORES, *shape)[c]
             for i, (name, shape, _) in enumerate(out_shapes)}
            for c in range(NCORES)
        ]

    return run, sharded, in_names, out_shapes, mesh


def _get_runner():
    global _RUNNER
    if _RUNNER is None:
        _RUNNER = _make_runner()
    return _RUNNER


def _in_maps(x, w_attn, b_attn, w_proj):
    maps = []
    for c in range(NCORES):
        b, j = c // 2, c % 2
        f0 = FH * j
        maps.append({
            "xb": np.ascontiguousarray(x[b]),
            "wq": np.ascontiguousarray(w_attn[:, f0:f0 + FH]),
            "wk": np.ascontiguousarray(w_attn[:, C + f0:C + f0 + FH]),
            "wv": np.ascontiguousarray(w_attn[:, 2 * C + f0:2 * C + f0 + FH]),
            "wp": np.ascontiguousarray(w_proj[f0:f0 + FH, :]),
            "bq": np.ascontiguousarray(b_attn[f0:f0 + FH]),
            "bk": np.ascontiguousarray(b_attn[C + f0:C + f0 + FH]),
            "bv": np.ascontiguousarray(b_attn[2 * C + f0:2 * C + f0 + FH]),
        })
    return maps


def kernel(x, w_attn, b_attn, w_proj, b_proj):
    x = np.asarray(x, dtype=np.float32)
    w_attn = np.asarray(w_attn, dtype=np.float32)
    b_attn = np.asarray(b_attn, dtype=np.float32)
    w_proj = np.asarray(w_proj, dtype=np.float32)
    b_proj = np.asarray(b_proj, dtype=np.float32)

    run, *_ = _get_runner()
    results = run(_in_maps(x, w_attn, b_attn, w_proj))

    y = np.empty((B, T, C), dtype=np.float32)
    for b in range(B):
        y[b] = results[2 * b]["yp"] + results[2 * b + 1]["yp"] + b_proj
    return y
